# revision 3
# baseline (speedup 1.0000x reference)
"""Causal single-head attention on 8 TRN2 NeuronCores — fp8 DoubleRow edition.

Problem: x [4, 2048, 768] f32; Wq/Wk/Wv [768, 768] f32 (torch Linear layout).
  q/k/v = x @ W.T ; scores = q k^T causal-masked; attn = softmax(scores/sqrt(768));
  out = attn @ v.

Sharding: core c -> batch b = c//2, half h = c%2; core h owns interleaved global
q-tiles {2*lt + h : lt in 0..7} (128 rows each), so both cores of a pair run the
identical SPMD instruction stream.

Scores are computed TRANSPOSED (S^T[keys, q]) so the exp output lands directly
in lhsT layout for the context matmul — no TensorE transposes. For local q-tile
lt the kernel computes n_kt = 2*lt+2 key tiles of 128 (exact causal rounded up
to an even count shared by both cores); the last two key tiles get a per-core
mask strip (triangle|full for h=0, zeros|triangle for h=1) added in PSUM before
the exp.

All heavy matmuls run in fp8e4 (e4m3) with MatmulPerfMode.DoubleRow: one
instruction contracts 256 (two 128-k-tiles) at 0.5 cycles per output row.
Precision is held by residual splits, with lo-residuals of the *weights*
pre-scaled by 16 on the host so they clear e4m3's subnormal floor; the matching
x/16 operand puts the product back on scale inside the same PSUM accumulation:
  proj(x, W) = x_hi@W_hi + x_lo@W_hi + (x/16)@(16*W_lo)      [Q, K, V]
The context uses V_hi + V_lo fp8 splits (V_lo applied on the diagonal key pair,
where attention concentrates), and the diagonal pair's attention weights also
get an fp8 lo-residual. Softmax renormalizes by the row-sum of the QUANTIZED
weights (ones-column appended to V_hi), cancelling common-mode fp8 error.
Emulated end-to-end precision of this scheme: rel err ~1.05e-2 (gate 2e-2).
"""

import os
import sys
from contextlib import ExitStack

import numpy as np

for _p in ("/opt/trn_rl_repo", "/root/.axon_site/_ro/trn_rl_repo"):
    if os.path.isdir(_p) and _p not in sys.path:
        sys.path.append(_p)

import ml_dtypes  # noqa: E402
import concourse.mybir as mybir  # noqa: E402
import concourse.tile as tile  # noqa: E402
from concourse import bacc  # noqa: E402
from concourse.bass_utils import run_bass_kernel_spmd  # noqa: E402

F32 = mybir.dt.float32
F8 = mybir.dt.float8e4
FP8NP = ml_dtypes.float8_e4m3
DR = mybir.MatmulPerfMode.DoubleRow
EXP = mybir.ActivationFunctionType.Exp
COPY = mybir.ActivationFunctionType.Copy

BATCH = 4
SEQ = 2048
D = 768
DK = D // 128  # 6 contraction k-tiles -> 3 DoubleRow pairs
NPAIR = DK // 2
NQ = 1024  # query rows per core
LT = NQ // 128  # local q-tiles per core (8)
XC = 512  # x streaming chunk width
NSC = SEQ // XC  # 4 chunks
SCALE = 1.0 / float(np.sqrt(np.float32(D)))
NEG = -1e30

_CACHE = {}


def _build():
    nc = bacc.Bacc("TRN2", target_bir_lowering=False, debug=False, num_devices=8)

    def dram(name, shape, dtype=F8, out=False):
        return nc.declare_dram_parameter(name, shape, dtype, isOutput=out)

    xh_d = dram("xh", [D, SEQ])
    xl_d = dram("xl", [D, SEQ])
    x16_d = dram("x16", [D, SEQ])
    xqh_d = dram("xqh", [D, NQ])
    xql_d = dram("xql", [D, NQ])
    xq16_d = dram("xq16", [D, NQ])
    wq_d = dram("wq", [D, D])
    wql_d = dram("wql", [D, D])  # 16*(Wq.T - hi)
    wk_d = dram("wk", [D, D])
    wkl_d = dram("wkl", [D, D])
    wvh_d = dram("wvh", [D, D])
    wvl_d = dram("wvl", [D, D])  # 16*(Wv.T - hi)
    strip_d = dram("strip", [128, 256], F32)
    out_d = dram("out", [NQ, D], F32, out=True)

    _dma_i = [0]

    def dma_in(dst, src):
        eng = (nc.sync, nc.gpsimd)[_dma_i[0] % 2]
        eng.dma_start(dst, src)
        _dma_i[0] += 1

    def rearr(dram_slice):
        return dram_slice.rearrange("(ko p) s -> p ko s", p=128)

    def load_w(dst, src_d):
        for half in range(2):
            dma_in(
                dst[:, half * 3 : half * 3 + 3, :],
                rearr(src_d[half * 384 : half * 384 + 384, :]),
            )

    with tile.TileContext(nc) as tc, ExitStack() as ctx:
        persist = ctx.enter_context(tc.tile_pool(name="persist", bufs=1))

        strip = persist.tile([128, 256], F32)
        nc.sync.dma_start(strip[:], strip_d[:])

        kt = persist.tile([128, DK, SEQ], F8)  # K^T
        qt = persist.tile([128, DK, NQ], F8)  # Q^T (own rows)
        vh = persist.tile([128, SEQ // 128, D + 1], F8)  # V hi + ones col
        vl = persist.tile([128, SEQ // 128, D], F8)  # V lo
        nc.gpsimd.memset(vh[:, :, D : D + 1], 1.0)  # softmax row-sum column

        # x chunks stay resident through the V phase
        xh = [persist.tile([128, DK, XC], F8, name=f"xh{i}") for i in range(NSC)]
        xl = [persist.tile([128, DK, XC], F8, name=f"xl{i}") for i in range(NSC)]
        x16 = [persist.tile([128, DK, XC], F8, name=f"x16{i}") for i in range(NSC)]

        with ExitStack() as wscope:
            wpool = wscope.enter_context(tc.tile_pool(name="wpool", bufs=1))
            xqpool = wscope.enter_context(tc.tile_pool(name="xqpool", bufs=1))

            wq = wpool.tile([128, DK, D], F8, name="wq")
            wql = wpool.tile([128, DK, D], F8, name="wql")
            wk = wpool.tile([128, DK, D], F8, name="wk")
            wkl = wpool.tile([128, DK, D], F8, name="wkl")
            wvh = wpool.tile([128, DK, D], F8, name="wvh")
            wvl = wpool.tile([128, DK, D], F8, name="wvl")
            nqh = NQ // XC  # 2
            xqh = [xqpool.tile([128, DK, XC], F8, name=f"xqh{i}") for i in range(nqh)]
            xql = [xqpool.tile([128, DK, XC], F8, name=f"xql{i}") for i in range(nqh)]
            xq16 = [
                xqpool.tile([128, DK, XC], F8, name=f"xq16{i}") for i in range(nqh)
            ]

            # DMA order is consumption order (phases run V, Q, K, A); two
            # queues drain it in parallel with compute.
            load_w(wvh, wvh_d)
            dma_in(xh[0][:], rearr(xh_d[:, 0:XC]))
            dma_in(xl[0][:], rearr(xl_d[:, 0:XC]))
            load_w(wvl, wvl_d)
            dma_in(x16[0][:], rearr(x16_d[:, 0:XC]))
            for i in range(1, NSC):
                dma_in(xh[i][:], rearr(xh_d[:, i * XC : (i + 1) * XC]))
                dma_in(xl[i][:], rearr(xl_d[:, i * XC : (i + 1) * XC]))
                dma_in(x16[i][:], rearr(x16_d[:, i * XC : (i + 1) * XC]))
            load_w(wq, wq_d)
            for i in range(nqh):
                dma_in(xqh[i][:], rearr(xqh_d[:, i * XC : (i + 1) * XC]))
                dma_in(xql[i][:], rearr(xql_d[:, i * XC : (i + 1) * XC]))
                if i == 0:
                    load_w(wql, wql_d)
                dma_in(xq16[i][:], rearr(xq16_d[:, i * XC : (i + 1) * XC]))
            load_w(wk, wk_d)
            load_w(wkl, wkl_d)

            def proj_group(ps_ap, stat_terms, oo):
                """3-term DoubleRow accumulation into ps_ap.
                stat_terms: [(w_tile, x_tile), ...]; lhsT = w[:, pair, oo*128:+128]."""
                first = (0, 0)
                last = (len(stat_terms) - 1, NPAIR - 1)
                for ti, (wt, xt_) in enumerate(stat_terms):
                    for j in range(NPAIR):
                        nc.tensor.matmul(
                            ps_ap,
                            wt[:, 2 * j : 2 * j + 2, oo * 128 : (oo + 1) * 128],
                            xt_[:, 2 * j : 2 * j + 2, :],
                            start=((ti, j) == first),
                            stop=((ti, j) == last),
                            perf_mode=DR,
                        )

            with ExitStack() as s1:
                ps_p = s1.enter_context(tc.tile_pool(name="ps_p", bufs=4, space="PSUM"))

                # ---------------- Phase V: V projection ----------------
                for sc in range(NSC):
                    for st in range(XC // 128):
                        seq_tile = sc * (XC // 128) + st
                        cs = st * 128
                        for oc in range(2):
                            pv_ps = ps_p.tile([128, XC], F32, tag="ps_p")
                            terms = [(xh[sc], wvh), (xl[sc], wvh), (x16[sc], wvl)]
                            first, last = (0, 0), (len(terms) - 1, NPAIR - 1)
                            for ti, (xt_, wt) in enumerate(terms):
                                for j in range(NPAIR):
                                    nc.tensor.matmul(
                                        pv_ps[:, 0:384],
                                        xt_[:, 2 * j : 2 * j + 2, cs : cs + 128],
                                        wt[:, 2 * j : 2 * j + 2, oc * 384 : (oc + 1) * 384],
                                        start=((ti, j) == first),
                                        stop=((ti, j) == last),
                                        perf_mode=DR,
                                    )
                            # v_hi (ScalarE) then v_lo = v - v_hi (DVE)
                            dst = vh[:, seq_tile, oc * 384 : (oc + 1) * 384]
                            nc.scalar.copy(dst, pv_ps[:, 0:384])
                            nc.vector.tensor_sub(
                                vl[:, seq_tile, oc * 384 : (oc + 1) * 384],
                                pv_ps[:, 0:384],
                                dst,
                            )

                # ---------------- Phase Q: Q^T projection ----------------
                for sc in range(nqh):
                    for oo in range(DK):
                        pq_ps = ps_p.tile([128, XC], F32, tag="ps_p")
                        proj_group(
                            pq_ps[:],
                            [(wq, xqh[sc]), (wq, xql[sc]), (wql, xq16[sc])],
                            oo,
                        )
                        dst = qt[:, oo, sc * XC : (sc + 1) * XC]
                        if oo % 2 == 0:
                            nc.scalar.copy(dst, pq_ps[:])
                        else:
                            nc.vector.tensor_copy(dst, pq_ps[:])

                # ---------------- Phase K: K^T projection ----------------
                for sc in range(NSC):
                    for oo in range(DK):
                        pk_ps = ps_p.tile([128, XC], F32, tag="ps_p")
                        proj_group(
                            pk_ps[:],
                            [(wk, xh[sc]), (wk, xl[sc]), (wkl, x16[sc])],
                            oo,
                        )
                        dst = kt[:, oo, sc * XC : (sc + 1) * XC]
                        if oo % 2 == 0:
                            nc.vector.tensor_copy(dst, pk_ps[:])
                        else:
                            nc.scalar.copy(dst, pk_ps[:])

        # ---------------- Phase A: attention per local q-tile ----------------
        # Software-pipelined: scores/exp of q-tile lt+1 are emitted before the
        # context matmuls of q-tile lt, so the PE has score work queued while
        # ScalarE runs the exps the context depends on.
        with ExitStack() as pa:
            ps_s = pa.enter_context(tc.tile_pool(name="ps_s", bufs=4, space="PSUM"))
            ps_c = pa.enter_context(tc.tile_pool(name="ps_c", bufs=2, space="PSUM"))
            ah_p = pa.enter_context(tc.tile_pool(name="ah", bufs=2))
            alo_p = pa.enter_context(tc.tile_pool(name="alo", bufs=2))
            a32_p = pa.enter_context(tc.tile_pool(name="a32", bufs=2))
            ctx_p = pa.enter_context(tc.tile_pool(name="ctxs", bufs=3))
            small_p = pa.enter_context(tc.tile_pool(name="small", bufs=3))

            tiles = {}

            def emit_scores(lt):
                n = 2 * lt + 2  # key tiles (even, SPMD-uniform)
                nbank = (n + 3) // 4
                ah = ah_p.tile([128, 16, 128], F8, tag="ah")
                alo = alo_p.tile([128, 2, 128], F8, tag="alo")
                tiles[lt] = (ah, alo)

                for bk in range(nbank):
                    t0, t1 = 4 * bk, min(4 * bk + 4, n)
                    ps = ps_s.tile([128, 512], F32, tag="ps_s")
                    for t in range(t0, t1):
                        co = (t - t0) * 128
                        for j in range(NPAIR):
                            nc.tensor.matmul(
                                ps[:, co : co + 128],
                                kt[:, 2 * j : 2 * j + 2, t * 128 : (t + 1) * 128],
                                qt[:, 2 * j : 2 * j + 2, lt * 128 : (lt + 1) * 128],
                                start=(t == t0 and j == 0),
                                stop=(t == t1 - 1 and j == NPAIR - 1),
                                perf_mode=DR,
                            )
                    if bk == nbank - 1:
                        # diagonal pair: mask strip; one exp covers the whole
                        # bank into fp8 ah (incl. diagonal), a second f32 exp
                        # of just the diagonal feeds the lo residual.
                        c0 = ((n - 2) - t0) * 128
                        nc.vector.tensor_add(
                            ps[:, c0 : c0 + 256], ps[:, c0 : c0 + 256], strip[:]
                        )
                        nc.scalar.activation(
                            ah[:, t0:t1, :],
                            ps[:, 0 : c0 + 256],
                            EXP,
                            scale=SCALE,
                        )
                        a32 = a32_p.tile([128, 256], F32, tag="a32")
                        nc.scalar.activation(
                            a32[:], ps[:, c0 : c0 + 256], EXP, scale=SCALE
                        )
                        nc.vector.tensor_sub(alo[:], a32[:], ah[:, n - 2 : n, :])
                    else:
                        nc.scalar.activation(
                            ah[:, t0:t1, :], ps[:], EXP, scale=SCALE
                        )

            def emit_ctx(lt):
                n = 2 * lt + 2
                ah, alo = tiles.pop(lt)
                # ctx[q, 0:769] in one 2-bank psum tile (col 768 = rowsum).
                # The rowsum bank (cols 512:769) is emitted FIRST so the DVE
                # reciprocal overlaps the first bank's matmuls.
                pc = ps_c.tile([128, 1024], F32, tag="ps_c")
                for pcols in (slice(512, 769), slice(0, 512)):
                    vlc = slice(pcols.start, min(pcols.stop, D))
                    passes = [("ah", 2 * u, vh, pcols) for u in range(n // 2)]
                    passes.append(("alo", n - 2, vh, pcols))
                    passes.append(("ah", n - 2, vl, vlc))
                    for pi, (akind, u0, vsrc, vc) in enumerate(passes):
                        lhs = alo[:, 0:2, :] if akind == "alo" else ah[:, u0 : u0 + 2, :]
                        nc.tensor.matmul(
                            pc[:, pcols.start : pcols.start + (vc.stop - vc.start)],
                            lhs,
                            vsrc[:, u0 : u0 + 2, vc],
                            start=(pi == 0),
                            stop=(pi == len(passes) - 1),
                            perf_mode=DR,
                        )
                    if pcols.start == 512:
                        rinv = small_p.tile([128, 1], F32, tag="rinv")
                        nc.vector.reciprocal(rinv[:], pc[:, 768:769])

                ctx_sb = ctx_p.tile([128, D], F32, tag="ctxs")
                nc.vector.tensor_mul(
                    ctx_sb[:], pc[:, 0:768], rinv[:].to_broadcast((128, D))
                )
                nc.sync.dma_start(out_d[lt * 128 : (lt + 1) * 128, :], ctx_sb[:])

            for lt in range(LT):
                emit_scores(lt)
                if lt >= 1:
                    emit_ctx(lt - 1)
            emit_ctx(LT - 1)

    nc.compile()
    return nc


def kernel(x, Wq, Wk, Wv):
    if "nc" not in _CACHE:
        _CACHE["nc"] = _build()
    nc = _CACHE["nc"]

    x = np.asarray(x, dtype=np.float32)

    def split_w(W):
        wt = np.asarray(W, dtype=np.float32).T
        hi = wt.astype(FP8NP)
        lo = (16.0 * (wt - hi.astype(np.float32))).astype(FP8NP)
        return np.ascontiguousarray(hi), np.ascontiguousarray(lo)

    wq8, wql8 = split_w(Wq)
    wk8, wkl8 = split_w(Wk)
    wv8, wvl8 = split_w(Wv)

    # strip: [tri | NEG] for h=0, [zeros | tri] for h=1
    i = np.arange(128)[:, None]
    j = np.arange(128)[None, :]
    tri = np.where(i <= j, 0.0, NEG).astype(np.float32)
    full = np.full((128, 128), NEG, np.float32)
    zeros = np.zeros((128, 128), np.float32)
    strips = [
        np.ascontiguousarray(np.concatenate([tri, full], axis=1)),
        np.ascontiguousarray(np.concatenate([zeros, tri], axis=1)),
    ]

    in_maps = []
    for c in range(8):
        b, h = c // 2, c % 2
        xt = x[b].T  # [d, seq]
        xh8 = xt.astype(FP8NP)
        xl8 = (xt - xh8.astype(np.float32)).astype(FP8NP)
        x168 = (xt / 16.0).astype(FP8NP)
        qcols = np.concatenate(
            [np.arange((2 * lt + h) * 128, (2 * lt + h + 1) * 128) for lt in range(LT)]
        )
        in_maps.append(
            {
                "xh": np.ascontiguousarray(xh8),
                "xl": np.ascontiguousarray(xl8),
                "x16": np.ascontiguousarray(x168),
                "xqh": np.ascontiguousarray(xh8[:, qcols]),
                "xql": np.ascontiguousarray(xl8[:, qcols]),
                "xq16": np.ascontiguousarray(x168[:, qcols]),
                "wq": wq8,
                "wql": wql8,
                "wk": wk8,
                "wkl": wkl8,
                "wvh": wv8,
                "wvl": wvl8,
                "strip": strips[h],
            }
        )

    res = run_bass_kernel_spmd(
        nc,
        in_maps,
        list(range(8)),
        trace=bool(int(os.environ.get("KERNEL_TRACE", "0"))),
    )
    _CACHE["last_results"] = res

    out = np.empty((BATCH, SEQ, D), np.float32)
    for c in range(8):
        b, h = c // 2, c % 2
        o = res.results[c]["out"]
        for lt in range(LT):
            out[b, (2 * lt + h) * 128 : (2 * lt + h + 1) * 128] = o[
                lt * 128 : (lt + 1) * 128
            ]
    return out
